# revision 26
# baseline (speedup 1.0000x reference)
"""Trainium2 Bass kernel for PVT-style spatial-reduction cross-attention.

Problem shapes (hardcoded): x [4, 3136, 512], v [4, 3136, 512], h=w=56,
8 heads (dh=64), sr_ratio=2 -> 784 kv tokens, fp32 I/O.

Sharding: 8 cores = 4 batches x 2 query-token halves. Each core computes
the full conv+LN+kv path for its batch (duplicated within the pair) and
attention + output projection for its 1568 query tokens. No collectives.

Layout strategy: activations are kept channel-major ("transposed") on chip
so every matmul contracts over the partition dim. The host supplies x^T and
v^T (layout choice during sharding). Scores are computed transposed
([ktok, qtok]); softmax denominators come from a ones-column appended to
the value matrix; normalization is deferred through the attention-output
matmul and applied via a gpsimd partition-broadcast of 1/denom.

Performance structure (345us baseline -> ~205us):
 - DMA order: q-path inputs first, all four q-projections issued up front
   to fill the input-DMA window; x^T persists in SBUF.
 - phases fused per kv-chunk: conv -> LN -> transpose -> vv projection;
   kT projection in two batched halves.
 - attention: head pairs (2p, 2p+1) share kT/qT tiles at partition offsets
   0/64, so their score matmuls occupy disjoint PE row groups and run
   concurrently; both write one 2-bank PSUM tile and a single ACT exp
   instruction covers [112, 2*qn], halving ACT instruction overhead (the
   exp stream on the Scalar engine is the kernel's critical path).
 - the 16 (q-chunk, head-pair) blocks form one flat software-pipelined
   stream (scores one block ahead of attention) - no q-chunk boundaries.
 - softmax denominators: DVE copy (not ACT), gpsimd partition-broadcast
   (ucode library pre-loaded at startup - the lazy LOAD_LIB costs ~7us
   otherwise), and reciprocal_approx_fast (~5x cheaper than reciprocal).
 - all phase-4 SBUF-shuffle/output DMAs are deferred two pipeline blocks:
   dma_start semaphore waits execute in-order ON the Sync engine, and an
   early-issued DMA whose producer has not retired stalls every later
   cross-engine notify behind it.
"""

import functools
import sys

import numpy as np

try:
    import concourse.bass as bass
except ImportError:  # pragma: no cover
    sys.path.insert(0, "/opt/trn_rl_repo")
    import concourse.bass as bass

import ml_dtypes
from concourse import bass_utils, mybir
from concourse.masks import make_identity
from concourse.tile import TileContext

BF16 = mybir.dt.bfloat16
F32 = mybir.dt.float32
NPBF = ml_dtypes.bfloat16

P = 128
C = 512          # channels
NH = 8           # heads
DH = 64          # head dim
B = 4
HH = 56
WW = 56
N = HH * WW      # 3136 query tokens per batch
NQ = N // 2      # 1568 query tokens per core
NKV = 784        # kv tokens per batch (28x28)
MKV = 112        # kv-token chunk (4 rows of 28)
NKV_CH = NKV // MKV  # 7
KC = C // P      # 4 channel chunks
SCALE = (C // NH) ** -0.5  # 0.125
EPS = 1e-5
QCH = [(0, 512), (512, 512), (1024, 512), (1536, 32)]  # query chunks per core

AluOp = mybir.AluOpType
Act = mybir.ActivationFunctionType


def build_nc():
    from concourse import bacc
    nc = bacc.Bacc()

    xT_d = nc.dram_tensor("xT", [C, NQ], BF16, kind="ExternalInput")
    # v pre-gathered on host into conv-slice layout: [di, dj, c, kvtok]
    vT_d = nc.dram_tensor("vT", [2, 2, C, NKV], BF16, kind="ExternalInput")
    qw_d = nc.dram_tensor("q_w", [C, C], BF16, kind="ExternalInput")
    kvw_d = nc.dram_tensor("kv_w", [C, 2 * C], BF16, kind="ExternalInput")
    srw_d = nc.dram_tensor("sr_w", [2, 2, C, C], BF16, kind="ExternalInput")
    projw_d = nc.dram_tensor("proj_w", [C, C], BF16, kind="ExternalInput")
    srb_d = nc.dram_tensor("sr_b", [P, C], F32, kind="ExternalInput")
    vvb_d = nc.dram_tensor("vv_b", [P, C], F32, kind="ExternalInput")
    kvbk_d = nc.dram_tensor("kv_bk", [P, KC], F32, kind="ExternalInput")
    projb_d = nc.dram_tensor("proj_b", [P, C], F32, kind="ExternalInput")
    y_d = nc.dram_tensor("y", [NQ, C], F32, kind="ExternalOutput")

    with TileContext(nc) as tc:
        from contextlib import ExitStack

        with ExitStack() as ctx:
            const = ctx.enter_context(tc.tile_pool(name="const", bufs=1))

            # ---- DMA order: q path first (smallest, unblocks qproj),
            # then conv inputs; proj weights last ----
            qw_sb = []
            xt_sb = []
            for kc in range(KC):
                t = const.tile([P, C], BF16, tag=f"qw{kc}", name=f"qw{kc}")
                nc.sync.dma_start(out=t, in_=qw_d[kc * P:(kc + 1) * P, :])
                qw_sb.append(t)
                t = const.tile([P, NQ], BF16, tag=f"xt{kc}", name=f"xt{kc}")
                nc.sync.dma_start(out=t, in_=xT_d[kc * P:(kc + 1) * P, :])
                xt_sb.append(t)
            srw_sb = {}
            for di in range(2):
                for dj in range(2):
                    for kc in range(KC):
                        t = const.tile([P, C], BF16, tag=f"srw{di}{dj}{kc}", name=f"srw{di}{dj}{kc}")
                        nc.sync.dma_start(
                            out=t, in_=srw_d[di, dj, kc * P:(kc + 1) * P, :]
                        )
                        srw_sb[(di, dj, kc)] = t
            vt_sb = {}
            for di in range(2):
                for dj in range(2):
                    for kc in range(KC):
                        t = const.tile([P, NKV], BF16, tag=f"vt{di}{dj}{kc}",
                                       name=f"vt{di}{dj}{kc}")
                        nc.sync.dma_start(
                            out=t, in_=vT_d[di, dj, kc * P:(kc + 1) * P, :]
                        )
                        vt_sb[(di, dj, kc)] = t
            srb_bc = const.tile([P, C], F32, tag="srb", name="srb")
            nc.sync.dma_start(out=srb_bc, in_=srb_d[:])
            kvw_sb = []
            for kc in range(KC):
                t = const.tile([P, 2 * C], BF16, tag=f"kvw{kc}", name=f"kvw{kc}")
                nc.sync.dma_start(out=t, in_=kvw_d[kc * P:(kc + 1) * P, :])
                kvw_sb.append(t)
            vvb_bc = const.tile([P, C], F32, tag="vvb", name="vvb")
            nc.sync.dma_start(out=vvb_bc, in_=vvb_d[:])
            # k-part of kv bias, per-partition layout [128, 4] (chunk j in col j)
            kbias_sb = const.tile([P, KC], F32, tag="kb", name="kb")
            nc.sync.dma_start(out=kbias_sb, in_=kvbk_d[:])
            projw_sb = []
            for kc in range(KC):
                t = const.tile([P, C], BF16, tag=f"pw{kc}", name=f"pw{kc}")
                nc.sync.dma_start(out=t, in_=projw_d[kc * P:(kc + 1) * P, :])
                projw_sb.append(t)
            projb_bc = const.tile([P, C], F32, tag="pjb", name="pjb")
            nc.sync.dma_start(out=projb_bc, in_=projb_d[:])

            ident = const.tile([P, P], BF16, tag="ident", name="ident")
            make_identity(nc, ident)

            zero_ap = const.tile([P, 1], F32, tag="zconst", name="zconst")
            nc.vector.memset(zero_ap, 0.0)
            nc.const_aps.aps[(F32, 0.0)] = zero_ap[:]
            eps_ap = const.tile([P, 1], F32, tag="epsconst", name="epsconst")
            nc.vector.memset(eps_ap, EPS)

            # Pre-load the gpsimd partition_broadcast ucode library during
            # the startup DMA window: the lazy LOAD_LIB otherwise costs
            # ~7us right when phase 4's first softmax denominator needs it.
            gpw_in = const.tile([1, 8], F32, tag="gpwi", name="gpwi")
            nc.vector.memset(gpw_in, 1.0)
            gpw_out = const.tile([2, 8], F32, tag="gpwo", name="gpwo")
            nc.gpsimd.partition_broadcast(gpw_out[:], gpw_in[:])

            # persistent activations
            kvT_sb = [const.tile([P, NKV], BF16, tag=f"kvt{j}", name=f"kvt{j}") for j in range(KC)]
            kT_sb = [const.tile([P, NKV], BF16, tag=f"kt{j}", name=f"kt{j}") for j in range(KC)]
            vv_sb = [
                const.tile([P, NH * (DH + 1)], BF16, tag=f"vv{m}", name=f"vv{m}")
                for m in range(NKV_CH)
            ]
            qT_sb = [const.tile([P, NQ], BF16, tag=f"qt{j}", name=f"qt{j}") for j in range(KC)]

            expp = ctx.enter_context(tc.tile_pool(name="expp", bufs=22))
            early_exps = {}

            # ------- fused phase 1+2+3: conv/LN/transpose/vv + kT + q -------
            with tc.tile_pool(name="w1", bufs=2) as w1, \
                 tc.tile_pool(name="ps_cv", bufs=2, space="PSUM") as ps_cv, \
                 tc.tile_pool(name="ps_tp", bufs=2, space="PSUM") as ps_tp, \
                 tc.tile_pool(name="ps_e", bufs=2, space="PSUM") as ps_e, \
                 tc.tile_pool(name="ps_mm", bufs=2, space="PSUM") as ps_mm:
                xn_tiles = {}

                def conv_ln(m):
                    conv_ps = ps_cv.tile([P, C], F32, tag="conv", name="conv")
                    nmm = 0
                    for di in range(2):
                        for dj in range(2):
                            for kc in range(KC):
                                nc.tensor.matmul(
                                    conv_ps[:MKV, :],
                                    vt_sb[(di, dj, kc)][:, m * MKV:(m + 1) * MKV],
                                    srw_sb[(di, dj, kc)][:],
                                    start=(nmm == 0),
                                    stop=(nmm == 15),
                                )
                                nmm += 1
                    # LayerNorm over the free dim (channels)
                    x_c = w1.tile([P, C], F32, tag="lnx", name="lnx")
                    nc.vector.tensor_tensor(
                        x_c[:MKV], conv_ps[:MKV], srb_bc[:MKV], AluOp.add
                    )
                    sums = w1.tile([P, 1], F32, tag="lnsum", name="lnsum")
                    nc.vector.reduce_sum(
                        out=sums[:MKV], in_=x_c[:MKV], axis=mybir.AxisListType.X
                    )
                    mu = w1.tile([P, 1], F32, tag="lnmu", name="lnmu")
                    nc.vector.tensor_scalar_mul(mu[:MKV], sums[:MKV], 1.0 / C)
                    sq_scr = w1.tile([P, C], BF16, tag="lnsq", name="lnsq")
                    sqs = w1.tile([P, 1], F32, tag="lnsqs", name="lnsqs")
                    nc.scalar.activation(
                        sq_scr[:MKV], x_c[:MKV], Act.Square, accum_out=sqs[:MKV]
                    )
                    mu2 = w1.tile([P, 1], F32, tag="lnmu2", name="lnmu2")
                    nc.vector.tensor_tensor(mu2[:MKV], mu[:MKV], mu[:MKV], AluOp.mult)
                    var = w1.tile([P, 1], F32, tag="lnvar", name="lnvar")
                    nc.vector.tensor_scalar(
                        var[:MKV], sqs[:MKV], 1.0 / C, None, AluOp.mult
                    )
                    nc.vector.tensor_tensor(var[:MKV], var[:MKV], mu2[:MKV], AluOp.subtract)
                    std = w1.tile([P, 1], F32, tag="lnstd", name="lnstd")
                    nc.scalar.activation(std[:MKV], var[:MKV], Act.Sqrt, bias=eps_ap[:MKV])
                    rstd = w1.tile([P, 1], F32, tag="lnrstd", name="lnrstd")
                    nc.vector.reciprocal_approx_fast(rstd[:MKV], std[:MKV])
                    xn = w1.tile([P, C], BF16, tag="lnout", name="lnout")
                    nc.vector.tensor_scalar(
                        xn[:MKV], x_c[:MKV], mu[:MKV], rstd[:MKV],
                        AluOp.subtract, AluOp.mult,
                    )
                    xn_tiles[m] = xn

                def tp_chunk(m):
                    # transpose [112, 512] -> kvT chunks [128, 112]
                    xn = xn_tiles.pop(m)
                    for j in range(KC):
                        tp_ps = ps_tp.tile([P, MKV], BF16, tag="tp", name="tp")
                        nc.tensor.transpose(
                            tp_ps[:, :MKV],
                            xn[:MKV, j * P:(j + 1) * P],
                            ident[:MKV, :MKV],
                        )
                        nc.vector.tensor_copy(
                            kvT_sb[j][:, m * MKV:(m + 1) * MKV], tp_ps[:, :MKV]
                        )

                def vv_chunk(m):
                    vv_ps = ps_mm.tile([P, C], F32, tag="mm", name="vvp")
                    for kc in range(KC):
                        nc.tensor.matmul(
                            vv_ps[:MKV, :],
                            kvT_sb[kc][:, m * MKV:(m + 1) * MKV],
                            kvw_sb[kc][:, C:],
                            start=(kc == 0),
                            stop=(kc == KC - 1),
                        )
                    vv_view = vv_sb[m].rearrange("p (h d) -> p h d", d=DH + 1)
                    nc.vector.tensor_tensor(
                        vv_view[:MKV, :, 0:DH],
                        vv_ps[:MKV].rearrange("p (h d) -> p h d", d=DH),
                        vvb_bc[:MKV].rearrange("p (h d) -> p h d", d=DH),
                        AluOp.add,
                    )
                    nc.vector.memset(vv_view[:MKV, :, DH:DH + 1], 1.0)

                def kt_part(n0, nn):
                    # k^T[:, n0:n0+nn]: [outc, ktok] = kv_w[:, :512]^T @ kv_^T
                    for j in range(KC):
                        kt_ps = ps_mm.tile([P, C], F32, tag="mm", name="ktp")
                        for kc in range(KC):
                            nc.tensor.matmul(
                                kt_ps[:, :nn],
                                kvw_sb[kc][:, j * P:(j + 1) * P],
                                kvT_sb[kc][:, n0:n0 + nn],
                                start=(kc == 0),
                                stop=(kc == KC - 1),
                            )
                        nc.vector.tensor_scalar_add(
                            kT_sb[j][:, n0:n0 + nn], kt_ps[:, :nn],
                            kbias_sb[:, j:j + 1],
                        )

                def sc_early(p, m):
                    # qc0 scores for kv-chunks 0..3 (they only need the first
                    # kT half), emitted inside the fused-phase tail from a
                    # dedicated PSUM pool so ACT starts the exp pipeline
                    # ~12us before the PE finishes phase 1-3.
                    q0, qn = QCH[0]
                    e = expp.tile([P, 1024], BF16, tag="expt", name="expt")
                    for s, hb in ((0, 0), (1, DH)):
                        scp = ps_e.tile([P, C], F32, tag="sce", name="sce")
                        nc.tensor.matmul(
                            scp[:MKV, :qn],
                            kT_sb[p][hb:hb + DH, m * MKV:(m + 1) * MKV],
                            qT_sb[p][hb:hb + DH, q0:q0 + qn],
                            start=True,
                            stop=True,
                        )
                        nc.scalar.activation(
                            e[:MKV, s * 512:s * 512 + qn], scp[:MKV, :qn],
                            Act.Exp, scale=SCALE,
                        )
                    early_exps[(p, m)] = e

                def qproj(q0, qn):
                    for j in range(KC):
                        qp_ps = ps_mm.tile([P, C], F32, tag="mm", name="qp")
                        for kc in range(KC):
                            nc.tensor.matmul(
                                qp_ps[:, :qn],
                                qw_sb[kc][:, j * P:(j + 1) * P],
                                xt_sb[kc][:, q0:q0 + qn],
                                start=(kc == 0),
                                stop=(kc == KC - 1),
                            )
                        nc.vector.tensor_copy(
                            qT_sb[j][:, q0:q0 + qn], qp_ps[:, :qn]
                        )

                qproj(*QCH[0])
                qproj(*QCH[1])
                qproj(*QCH[2])
                qproj(*QCH[3])
                conv_ln(0)
                conv_ln(1)
                tp_chunk(0)
                vv_chunk(0)
                conv_ln(2)
                tp_chunk(1)
                vv_chunk(1)
                conv_ln(3)
                tp_chunk(2)
                vv_chunk(2)
                conv_ln(4)
                tp_chunk(3)
                vv_chunk(3)
                kt_part(0, 448)
                sc_early(0, 0)
                conv_ln(5)
                sc_early(0, 1)
                tp_chunk(4)
                vv_chunk(4)
                sc_early(0, 2)
                conv_ln(6)
                sc_early(0, 3)
                tp_chunk(5)
                vv_chunk(5)
                sc_early(1, 0)
                tp_chunk(6)
                vv_chunk(6)
                sc_early(1, 1)
                kt_part(448, NKV - 448)
                sc_early(1, 2)
                sc_early(1, 3)

            # ---------------- Phase 4: attention + output projection ---------
            # Head pairs (2p, 2p+1) live at partition offsets 0/64 of kT/qT
            # tile p, so the pair's score matmuls hit disjoint PE row groups
            # and run concurrently into the two banks of one PSUM tile; one
            # exp instruction then covers both heads' scores.
            #
            # The 16 (q-chunk, pair) blocks form one flat software-pipelined
            # stream (scores run one block ahead of attn) so no bubble forms
            # at q-chunk boundaries. Sync-engine discipline: dma_start waits
            # execute in-order ON the Sync engine, so every SBUF-shuffle /
            # output DMA is queued and flushed a block later, when its
            # producer has retired.
            with tc.tile_pool(name="w4", bufs=4) as w4, \
                 tc.tile_pool(name="outp", bufs=3) as outp, \
                 tc.tile_pool(name="ps_sc", bufs=2, space="PSUM") as ps_sc, \
                 tc.tile_pool(name="ps_at", bufs=3, space="PSUM") as ps_at, \
                 tc.tile_pool(name="ps_pj", bufs=1, space="PSUM") as ps_pj:
                pend_dma = []
                pend_dma_old = []
                pend_proj = []
                qc_ot = {}

                def flush_dmas():
                    # two-block deferral: by the time the Sync engine reaches
                    # these, their producers have retired, so the in-order
                    # Sync queue never blocks on a long semaphore wait.
                    while pend_dma_old:
                        o, i = pend_dma_old.pop(0)
                        nc.sync.dma_start(out=o, in_=i)
                    pend_dma_old.extend(pend_dma)
                    del pend_dma[:]

                def do_scores_pair(qi, p):
                    q0, qn = QCH[qi]
                    exps = []
                    for m in range(NKV_CH):
                        if qi == 0 and (p, m) in early_exps:
                            exps.append(early_exps.pop((p, m)))
                            continue
                        sc2 = ps_sc.tile([P, 1024], F32, tag="sc2", name="sc2")
                        for s, hb in ((0, 0), (1, DH)):
                            nc.tensor.matmul(
                                sc2[:MKV, s * 512:s * 512 + qn],
                                kT_sb[p][hb:hb + DH, m * MKV:(m + 1) * MKV],
                                qT_sb[p][hb:hb + DH, q0:q0 + qn],
                                start=True,
                                stop=True,
                            )
                        e = expp.tile([P, 1024], BF16, tag="expt", name="expt")
                        if qn == 512:
                            nc.scalar.activation(
                                e[:MKV, :], sc2[:MKV, :], Act.Exp, scale=SCALE
                            )
                        else:
                            for s in (0, 1):
                                nc.scalar.activation(
                                    e[:MKV, s * 512:s * 512 + qn],
                                    sc2[:MKV, s * 512:s * 512 + qn],
                                    Act.Exp, scale=SCALE,
                                )
                        exps.append(e)
                    return exps

                def do_attn(qi, h, exps):
                    q0, qn = QCH[qi]
                    ot = qc_ot[qi]
                    s = h % 2
                    jj, hb = h // 2, s * DH
                    at_ps = ps_at.tile([P, 512], F32, tag="at", name="at")
                    for m in range(NKV_CH):
                        nc.tensor.matmul(
                            at_ps[:DH + 1, :qn],
                            vv_sb[m][:MKV, h * (DH + 1):(h + 1) * (DH + 1)],
                            exps[m][:MKV, s * 512:s * 512 + qn],
                            start=(m == 0),
                            stop=(m == NKV_CH - 1),
                        )
                    den = w4.tile([1, 512], F32, tag="den", name="den")
                    nc.vector.tensor_copy(den[:, :qn], at_ps[DH:DH + 1, :qn])
                    rb = w4.tile([DH, 512], F32, tag="rb", name="rb")
                    nc.gpsimd.partition_broadcast(rb[:, :qn], den[:, :qn])
                    nc.vector.reciprocal_approx_fast(rb[:, :qn], rb[:, :qn])
                    if hb == 0:
                        nc.vector.tensor_tensor(
                            ot[jj][0:DH, :qn], at_ps[0:DH, :qn], rb[:, :qn],
                            AluOp.mult,
                        )
                    else:
                        oddscr = w4.tile([DH, 512], BF16, tag="oddscr", name="oddscr")
                        nc.vector.tensor_tensor(
                            oddscr[:, :qn], at_ps[0:DH, :qn], rb[:, :qn],
                            AluOp.mult,
                        )
                        pend_dma.append((ot[jj][DH:2 * DH, :qn], oddscr[:, :qn]))

                def do_proj(state):
                    (pot, pq0, pqn, mq) = state
                    mqn = min(P, pqn - mq * P)
                    pj_ps = ps_pj.tile([P, C], F32, tag="pj", name="pj")
                    for j in range(KC):
                        nc.tensor.matmul(
                            pj_ps[:mqn, :],
                            pot[j][:, mq * P:mq * P + mqn],
                            projw_sb[j][:],
                            start=(j == 0),
                            stop=(j == KC - 1),
                        )
                    yb = w4.tile([P, C], F32, tag="yb", name="yb")
                    nc.vector.tensor_tensor(
                        yb[:mqn], pj_ps[:mqn], projb_bc[:mqn], AluOp.add
                    )
                    pend_dma.append(
                        (y_d[pq0 + mq * P:pq0 + mq * P + mqn, :], yb[:mqn])
                    )

                pend_proj_stage = []

                def retire(block):
                    # staged two deep so a q-chunk's projections only pop
                    # after the flush that issues its odd-head shift DMAs.
                    # Up to two pops per retire drains the backlog before the
                    # final block, keeping projections out of the end tail.
                    (qi, pp, ex) = block
                    for _ in range(2):
                        if pend_proj:
                            do_proj(pend_proj.pop(0))
                    pend_proj.extend(pend_proj_stage)
                    del pend_proj_stage[:]
                    do_attn(qi, 2 * pp, ex)
                    do_attn(qi, 2 * pp + 1, ex)
                    if pp == NH // 2 - 1:
                        q0, qn = QCH[qi]
                        for mq in range((qn + P - 1) // P):
                            pend_proj_stage.append((qc_ot[qi], q0, qn, mq))

                pend = []
                for qi in range(len(QCH)):
                    qc_ot[qi] = [
                        outp.tile([P, 512], BF16, tag=f"ot{j}", name=f"ot{j}")
                        for j in range(KC)
                    ]
                    for pp in range(NH // 2):
                        pend.append((qi, pp, do_scores_pair(qi, pp)))
                        flush_dmas()
                        if len(pend) > 1:
                            retire(pend.pop(0))
                retire(pend.pop(0))
                flush_dmas()
                flush_dmas()
                pend_proj.extend(pend_proj_stage)
                del pend_proj_stage[:]
                while pend_proj:
                    do_proj(pend_proj.pop(0))
                    flush_dmas()
                    flush_dmas()

    nc.finalize()
    return nc


@functools.lru_cache(maxsize=1)
def _get_nc():
    return build_nc()


def _prepare_in_maps(inputs):
    x = np.asarray(inputs["x"], dtype=np.float32)
    v = np.asarray(inputs["v"], dtype=np.float32)
    q_w = np.asarray(inputs["q_w"], dtype=np.float32)
    kv_w = np.asarray(inputs["kv_w"], dtype=np.float32)
    sr_w = np.asarray(inputs["sr_w"], dtype=np.float32)
    sr_b = np.asarray(inputs["sr_b"], dtype=np.float32)
    ln_g = np.asarray(inputs["ln_g"], dtype=np.float32)
    ln_b = np.asarray(inputs["ln_b"], dtype=np.float32)
    proj_w = np.asarray(inputs["proj_w"], dtype=np.float32)
    proj_b = np.asarray(inputs["proj_b"], dtype=np.float32)

    # fold LN affine into the kv projection: kv_w' = g[:,None]*kv_w,
    # kv_b' = b @ kv_w
    kvw_eff = (ln_g[:, None] * kv_w).astype(NPBF)
    kvb_eff = (ln_b @ kv_w).astype(np.float32)

    qw_bf = q_w.astype(NPBF)
    projw_bf = proj_w.astype(NPBF)
    srb_bc = np.ascontiguousarray(np.broadcast_to(sr_b, (P, C))).astype(np.float32)
    vvb_bc = np.ascontiguousarray(np.broadcast_to(kvb_eff[C:], (P, C))).astype(np.float32)
    kvbk = np.ascontiguousarray(kvb_eff[:C].reshape(KC, P).T).astype(np.float32)
    projb_bc = np.ascontiguousarray(np.broadcast_to(proj_b, (P, C))).astype(np.float32)
    # sr_w [O, I, kh, kw] -> [kh, kw, I, O]
    srw_t = np.ascontiguousarray(sr_w.transpose(2, 3, 1, 0)).astype(NPBF)

    in_maps = []
    vT_cache = {}
    for core in range(8):
        b, s = core // 2, core % 2
        if b not in vT_cache:
            # [56,56,C] -> [di,dj,C,28*28] conv-slice gather
            vb = v[b].reshape(28, 2, 28, 2, C).transpose(1, 3, 4, 0, 2)
            vT_cache[b] = np.ascontiguousarray(vb.reshape(2, 2, C, NKV)).astype(NPBF)
        xT = np.ascontiguousarray(x[b, s * NQ:(s + 1) * NQ, :].T).astype(NPBF)
        in_maps.append({
            "xT": xT,
            "vT": vT_cache[b],
            "q_w": qw_bf,
            "kv_w": kvw_eff,
            "sr_w": srw_t,
            "proj_w": projw_bf,
            "sr_b": srb_bc,
            "vv_b": vvb_bc,
            "kv_bk": kvbk,
            "proj_b": projb_bc,
        })

    return in_maps


def _assemble(results):
    out = np.empty((B, N, C), dtype=np.float32)
    for core in range(8):
        b, s = core // 2, core % 2
        out[b, s * NQ:(s + 1) * NQ, :] = results[core]["y"]
    return out


def kernel(**inputs) -> np.ndarray:
    in_maps = _prepare_in_maps(inputs)
    nc = _get_nc()
    res = bass_utils.run_bass_kernel_spmd(nc, in_maps, core_ids=list(range(8)))
    return _assemble(res.results)


if __name__ == "__main__":
    nc = build_nc()
    print("built ok")
